# revision 4
# baseline (speedup 1.0000x reference)
"""Trainium2 Bass kernel for nn_ClassificationHead.

Math (per batch b, query q):
  qe        = query_table[label_ids[b,q]]                      (E,)
  gamma,beta= split(qe @ W_film + b_film)                      (C,), (C,)
  film[n,c] = f[n,c]*gamma[c] + beta[c]      f = feature[b] as (HW, C)
  hidden    = gelu(film @ W_att_h + b_att_h)                   (HW, H)
  attn      = sigmoid(hidden @ W_att_f + b_att_f)              (HW, 1)
  pooled    = sum_n attn*film / (sum attn + 1e-8)              (C,)
  logits    = gelu(pooled @ W_mlp1 + b_mlp1) @ W_mlp2 + b_mlp2 (NCLS,)

Restructure used on device (film is never materialized):
  hiddenT[h,n] = gelu( sum_c (gamma[c]*W_att_h[c,h]) * fT[c,n] + bh[h] )
      with bh[h] = b_att_h[h] + sum_c beta[c]*W_att_h[c,h]
  attn = 0.5 + 0.5*u,  u[n] = tanh( 0.5*(hiddenT.T @ W_att_f)[n] + 0.5*b_att_f )
  pooled[c] = gamma[c]*(colsum_f[c] + t[c]) * a  +  beta[c]
      t[c] = sum_n f[n,c]*u[n],  a = 1/(HW + sum_n u[n] + 2e-8),
      colsum_f[c] = sum_n fT[c,n]
  (the beta ratio (HW+S)/(HW+S+2e-8) rounds to exactly 1.0 in fp32, as it
   does in the fp32 reference, so it is dropped)

Sharding: data-parallel over B across the 8 cores (B == 8).
Both gelu (hidden + MLP) and tanh come from the single ACT table set
`gelu_and_others`, so there is exactly one activation-table load.
"""

import os
import numpy as np

B, Q, C, H, W = 8, 512, 64, 16, 16  # placeholder reorder below (kept explicit)
B = 8
Q = 64
C = 512
HW = 256
E = 512
HID = 128       # HID_ATT
HID2 = 512      # HID_MLP
NCLS = 14
NLAB = 64
NCORES = 8

_CACHE = {}
LAST_EXEC_NS = None
LAST_TRACE = None


def _build_nc():
    import concourse.bass as bass
    import concourse.tile as tile
    from concourse import bacc, mybir

    f32 = mybir.dt.float32
    bf16 = mybir.dt.bfloat16
    A = mybir.ActivationFunctionType
    AX = mybir.AxisListType

    nc = bacc.Bacc("TRN2", target_bir_lowering=False, debug=False,
                   num_devices=NCORES)

    # ---- DRAM tensors (per-core inputs) ----
    feat_d = nc.dram_tensor("feat", [C, HW], f32, kind="ExternalInput")
    oh_d = nc.dram_tensor("onehot", [NLAB, Q], f32, kind="ExternalInput")
    qt_d = nc.dram_tensor("qt", [NLAB, E], f32, kind="ExternalInput")
    wfilm_d = nc.dram_tensor("wfilm", [E, 2 * C], f32, kind="ExternalInput")
    bfilm_d = nc.dram_tensor("bfilm", [1, 2 * C], f32, kind="ExternalInput")
    wah_d = nc.dram_tensor("wah", [C, HID], f32, kind="ExternalInput")
    bah_d = nc.dram_tensor("bah", [1, HID], f32, kind="ExternalInput")
    waf_d = nc.dram_tensor("waf", [HID, 1], f32, kind="ExternalInput")
    baf_d = nc.dram_tensor("baf", [1, 1], f32, kind="ExternalInput")
    wm1_d = nc.dram_tensor("wm1", [C, HID2], f32, kind="ExternalInput")
    bm1_d = nc.dram_tensor("bm1", [HID2], f32, kind="ExternalInput")
    wm2_d = nc.dram_tensor("wm2", [C, NCLS], f32, kind="ExternalInput")
    bm2_d = nc.dram_tensor("bm2", [1, NCLS], f32, kind="ExternalInput")
    id_d = nc.dram_tensor("ident", [128, 128], bf16, kind="ExternalInput")
    out_d = nc.dram_tensor("out", [Q, NCLS], f32, kind="ExternalOutput")

    KC = C // 128   # 4 c-chunks
    KN = HW // 128  # 2 n-chunks

    with tile.TileContext(nc) as tc:
        with (
            tc.tile_pool(name="const", bufs=1) as const,
            tc.tile_pool(name="work", bufs=3) as work,
            tc.tile_pool(name="psmisc", bufs=3, space="PSUM") as psm,
            tc.tile_pool(name="pshid", bufs=2, space="PSUM") as psh,
            tc.tile_pool(name="psz", bufs=2, space="PSUM") as psz,
        ):
            # ---------------- DMAs in ----------------
            qt_sb = const.tile([NLAB, E], f32)
            nc.sync.dma_start(qt_sb[:], qt_d[:])
            oh_sb = const.tile([NLAB, Q], f32)
            nc.sync.dma_start(oh_sb[:], oh_d[:])
            feat_sb = const.tile([128, KC, HW], f32)
            nc.sync.dma_start(
                feat_sb[:], feat_d.ap().rearrange("(k p) n -> p k n", p=128))
            wf_sb = []
            for k in range(KC):
                t = const.tile([128, 2 * C], f32, tag=f"wf{k}")
                nc.sync.dma_start(t[:], wfilm_d[k * 128:(k + 1) * 128, :])
                wf_sb.append(t)
            bfilm_sb = const.tile([1, 2 * C], f32)
            nc.sync.dma_start(bfilm_sb[:], bfilm_d[:])
            wah_sb = const.tile([128, KC, HID], f32)
            nc.sync.dma_start(
                wah_sb[:], wah_d.ap().rearrange("(k p) h -> p k h", p=128))
            bah_sb = const.tile([1, HID], f32)
            nc.sync.dma_start(bah_sb[:], bah_d[:])
            waf_sb = const.tile([HID, 1], f32)
            nc.sync.dma_start(waf_sb[:], waf_d[:])
            baf_sb = const.tile([1, 1], f32)
            nc.sync.dma_start(baf_sb[:], baf_d[:])
            wm1_sb = const.tile([128, KC, HID2], f32)
            nc.sync.dma_start(
                wm1_sb[:], wm1_d.ap().rearrange("(k p) m -> p k m", p=128))
            bm1_sb = const.tile([128, KC], f32)
            nc.sync.dma_start(
                bm1_sb[:], bm1_d.ap().rearrange("(k p) -> p k", p=128))
            wm2_sb = const.tile([128, KC, NCLS], f32)
            nc.sync.dma_start(
                wm2_sb[:], wm2_d.ap().rearrange("(k p) o -> p k o", p=128))
            bm2_sb = const.tile([1, NCLS], f32)
            nc.sync.dma_start(bm2_sb[:], bm2_d[:])
            id_sb = const.tile([128, 128], bf16)
            nc.sync.dma_start(id_sb[:], id_d[:])

            # ---------------- constants ----------------
            ones_row = const.tile([1, 128], f32)
            nc.vector.memset(ones_row[:], 1.0)
            half_row = const.tile([1, 128], f32)
            nc.vector.memset(half_row[:], 0.5)
            ones_col_b = const.tile([128, 1], bf16)
            nc.vector.memset(ones_col_b[:], 1.0)

            # ---------------- setup compute ----------------
            # bf16 copies of f^T and W_att_h; 0.5*W_att_f in bf16
            ftb = const.tile([128, KC, HW], bf16)
            nc.vector.tensor_copy(ftb[:], feat_sb[:])
            wahb = const.tile([128, KC, HID], bf16)
            nc.vector.tensor_copy(wahb[:], wah_sb[:])
            wafb = const.tile([HID, 1], bf16)
            nc.vector.tensor_scalar_mul(wafb[:], waf_sb[:], 0.5)

            # column sums of f^T (exact, fp32)
            colsum = const.tile([128, KC], f32)
            for k in range(KC):
                nc.vector.reduce_sum(colsum[:, k:k + 1], feat_sb[:, k, :],
                                     axis=AX.X)

            # f (HW-major, bf16) via SBUF->SBUF DMA transposes
            f_sb = []
            for i in range(KN):
                t = const.tile([128, C], bf16, tag=f"fsb{i}")
                f_sb.append(t)
            for i in range(KN):
                for k in range(KC):
                    nc.sync.dma_start_transpose(
                        f_sb[i][:, k * 128:(k + 1) * 128],
                        ftb[:, k, i * 128:(i + 1) * 128])

            # qe^T = query_table^T gathered by onehot: [e, q] in 4 chunks
            qeT = const.tile([128, KC, Q], f32)
            for k in range(KC):
                ps = psm.tile([128, Q], f32, tag="m")
                nc.tensor.matmul(ps[:], qt_sb[:, k * 128:(k + 1) * 128],
                                 oh_sb[:], start=True, stop=True)
                nc.vector.tensor_copy(qeT[:, k, :], ps[:])

            # gb^T = W_film^T @ qe^T + b_film  -> gammaT, betaT  [c, q]
            gammaT = const.tile([128, KC, Q], f32)
            betaT = const.tile([128, KC, Q], f32)
            for j in range(2 * KC):
                ps = psm.tile([128, Q], f32, tag="m")
                for k in range(KC):
                    nc.tensor.matmul(ps[:],
                                     wf_sb[k][:, j * 128:(j + 1) * 128],
                                     qeT[:, k, :],
                                     start=(k == 0), stop=False)
                nc.tensor.matmul(ps[:], bfilm_sb[:, j * 128:(j + 1) * 128],
                                 ones_row[:, :Q], start=False, stop=True)
                if j < KC:
                    nc.vector.tensor_copy(gammaT[:, j, :], ps[:])
                else:
                    nc.scalar.copy(betaT[:, j - KC, :], ps[:])

            # bh[h, q] = b_att_h[h] + sum_c beta[c,q] W_att_h[c,h]
            bh_sb = const.tile([HID, Q], f32)
            ps_bh = psm.tile([HID, Q], f32, tag="m")
            for k in range(KC):
                nc.tensor.matmul(ps_bh[:], wah_sb[:, k, :], betaT[:, k, :],
                                 start=(k == 0), stop=False)
            nc.tensor.matmul(ps_bh[:], bah_sb[:], ones_row[:, :Q],
                             start=False, stop=True)
            nc.vector.tensor_copy(bh_sb[:], ps_bh[:])

            # 0.5*b_att_f broadcast down 128 partitions
            bfa_half = const.tile([128, 1], f32)
            ps_bf = psm.tile([128, 1], f32, tag="m")
            nc.tensor.matmul(ps_bf[:], half_row[:], baf_sb[:],
                             start=True, stop=True)
            nc.vector.tensor_copy(bfa_half[:], ps_bf[:])

            # ---------------- main loop over queries ----------------
            # z rows are staged free-dim-packed on partition 0 (compute
            # engines cannot write at arbitrary partition offsets), then
            # repartitioned to [Q, HW] via SBUF->SBUF DMA in 4 stages.
            QS = Q // 4  # queries per repartition stage
            u_stage = const.tile([1, Q * HW], f32)
            u_all = const.tile([Q, HW], f32)
            for q in range(Q):
                wq = work.tile([128, KC, HID], bf16, tag="wq")
                for k in range(KC):
                    nc.vector.tensor_scalar_mul(
                        wq[:, k, :], wahb[:, k, :], gammaT[:, k, q:q + 1])
                hid_ps = psh.tile([HID, HW], f32, tag="h")
                for k in range(KC):
                    nc.tensor.matmul(hid_ps[:], wq[:, k, :], ftb[:, k, :],
                                     start=(k == 0), stop=(k == KC - 1))
                hidT = work.tile([HID, HW], bf16, tag="hidT")
                nc.scalar.activation(hidT[:], hid_ps[:], A.Gelu,
                                     bias=bh_sb[:, q:q + 1], scale=1.0)
                z_ps = psz.tile([1, HW], f32, tag="z")
                nc.tensor.matmul(z_ps[:], wafb[:], hidT[:],
                                 start=True, stop=True)
                # evacuate the z row (PSUM -> SBUF), alternating engines
                dst = u_stage[0:1, q * HW:(q + 1) * HW]
                if q % 2 == 0:
                    nc.scalar.copy(dst, z_ps[:])
                else:
                    nc.vector.tensor_copy(dst, z_ps[:])
                if q % QS == QS - 1:
                    s = q // QS
                    nc.sync.dma_start(
                        u_all[s * QS:(s + 1) * QS, :],
                        u_stage[0:1, s * QS * HW:(s + 1) * QS * HW])

            # ---------------- pooling tail ----------------
            # u = tanh(0.5 z + 0.5 b_att_f), bf16
            u_b = const.tile([Q, HW], bf16)
            nc.scalar.activation(u_b[:], u_all[:], A.Tanh,
                                 bias=bfa_half[:Q, :], scale=1.0)
            # u^T chunks [n, q] via PE transposes
            uT = const.tile([128, KN, Q], bf16)
            for i in range(KN):
                ps = psm.tile([128, Q], bf16, tag="m")
                nc.tensor.transpose(ps[:], u_b[:, i * 128:(i + 1) * 128],
                                    id_sb[:Q, :Q])
                nc.vector.tensor_copy(uT[:, i, :], ps[:])

            # t[c, q] = sum_n f[n,c] u[n,q];  S[q] = sum_n u[n,q]
            s_ps = psm.tile([1, Q], f32, tag="m")
            for i in range(KN):
                nc.tensor.matmul(s_ps[:], ones_col_b[:], uT[:, i, :],
                                 start=(i == 0), stop=(i == KN - 1))
            den = const.tile([1, Q], f32)
            nc.vector.tensor_scalar_add(den[:], s_ps[:], float(HW) + 2e-8)
            inv = const.tile([1, 128], f32)
            nc.vector.reciprocal(inv[:, :Q], den[:])
            # broadcast a = inv down partitions
            bc_ps = psm.tile([128, Q], f32, tag="m")
            nc.tensor.matmul(bc_ps[:], ones_row[:], inv[:, :Q],
                             start=True, stop=True)
            a_bc = const.tile([128, Q], f32)
            nc.vector.tensor_copy(a_bc[:], bc_ps[:])

            pooled = const.tile([128, KC, Q], f32)
            for k in range(KC):
                t_ps = psm.tile([128, Q], f32, tag="m")
                for i in range(KN):
                    nc.tensor.matmul(t_ps[:],
                                     f_sb[i][:, k * 128:(k + 1) * 128],
                                     uT[:, i, :],
                                     start=(i == 0), stop=(i == KN - 1))
                x1 = work.tile([128, Q], f32, tag="x1")
                nc.vector.tensor_scalar_add(x1[:], t_ps[:], colsum[:, k:k + 1])
                x2 = work.tile([128, Q], f32, tag="x2")
                nc.vector.tensor_mul(x2[:], x1[:], gammaT[:, k, :])
                x3 = work.tile([128, Q], f32, tag="x3")
                nc.vector.tensor_mul(x3[:], x2[:], a_bc[:])
                nc.vector.tensor_add(pooled[:, k, :], x3[:], betaT[:, k, :])

            # ---------------- classification MLP ----------------
            h2 = const.tile([128, KC, Q], f32)
            for j in range(KC):
                ps = psm.tile([128, Q], f32, tag="m")
                for k in range(KC):
                    nc.tensor.matmul(ps[:],
                                     wm1_sb[:, k, j * 128:(j + 1) * 128],
                                     pooled[:, k, :],
                                     start=(k == 0), stop=(k == KC - 1))
                nc.scalar.activation(h2[:, j, :], ps[:], A.Gelu,
                                     bias=bm1_sb[:, j:j + 1], scale=1.0)

            lg_ps = psm.tile([Q, NCLS], f32, tag="m")
            for j in range(KC):
                nc.tensor.matmul(lg_ps[:], h2[:, j, :], wm2_sb[:, j, :],
                                 start=(j == 0), stop=False)
            nc.tensor.matmul(lg_ps[:], ones_row[:, :Q], bm2_sb[:],
                             start=False, stop=True)
            lg_sb = const.tile([Q, NCLS], f32)
            nc.vector.tensor_copy(lg_sb[:], lg_ps[:])
            nc.sync.dma_start(out_d[:], lg_sb[:])

    nc.compile()
    return nc


def _maybe_install_trace_shim():
    """Register the NTFF profile hook (missing antenv.axon_hooks in this
    image) so run_bass_kernel_spmd(trace=True) can return exec_time_ns."""
    try:
        import sys, types
        import antenv  # noqa: F401
        if "antenv.axon_hooks" not in sys.modules:
            mod = types.ModuleType("antenv.axon_hooks")
            mod._hook = None
            def _set(h):
                mod._hook = h
            def _get():
                return mod._hook
            mod.set_axon_ntff_profile_hook = _set
            mod.get_axon_ntff_profile_hook = _get
            sys.modules["antenv.axon_hooks"] = mod
            antenv.axon_hooks = mod
        from trn_agent_boot.trn_boot import _ntff_profile_via_ctypes
        sys.modules["antenv.axon_hooks"].set_axon_ntff_profile_hook(
            _ntff_profile_via_ctypes("/opt/axon/libaxon_pjrt.so"))
        import concourse.bass_utils as bu
        bu.upload_artifacts = lambda tmpdir: tmpdir
        return True
    except Exception:
        return False


def kernel(**inputs) -> np.ndarray:
    global LAST_EXEC_NS, LAST_TRACE
    import ml_dtypes
    from concourse.bass_utils import run_bass_kernel_spmd

    feature = np.asarray(inputs["feature"], dtype=np.float32)      # (B,C,H,W)
    label_ids = np.asarray(inputs["label_ids"]).astype(np.int64)   # (B,Q)
    query_table = np.asarray(inputs["query_table"], dtype=np.float32)
    W_film = np.asarray(inputs["W_film"], dtype=np.float32)
    b_film = np.asarray(inputs["b_film"], dtype=np.float32)
    W_att_h = np.asarray(inputs["W_att_h"], dtype=np.float32)
    b_att_h = np.asarray(inputs["b_att_h"], dtype=np.float32)
    W_att_f = np.asarray(inputs["W_att_f"], dtype=np.float32)
    b_att_f = np.asarray(inputs["b_att_f"], dtype=np.float32)
    W_mlp1 = np.asarray(inputs["W_mlp1"], dtype=np.float32)
    b_mlp1 = np.asarray(inputs["b_mlp1"], dtype=np.float32)
    W_mlp2 = np.asarray(inputs["W_mlp2"], dtype=np.float32)
    b_mlp2 = np.asarray(inputs["b_mlp2"], dtype=np.float32)

    if "nc" not in _CACHE:
        _CACHE["nc"] = _build_nc()
    nc = _CACHE["nc"]

    ident = np.eye(128, dtype=ml_dtypes.bfloat16)
    lab_range = np.arange(NLAB, dtype=np.int64)

    shared = {
        "qt": np.ascontiguousarray(query_table),
        "wfilm": np.ascontiguousarray(W_film),
        "bfilm": np.ascontiguousarray(b_film.reshape(1, 2 * C)),
        "wah": np.ascontiguousarray(W_att_h),
        "bah": np.ascontiguousarray(b_att_h.reshape(1, HID)),
        "waf": np.ascontiguousarray(W_att_f.reshape(HID, 1)),
        "baf": np.ascontiguousarray(b_att_f.reshape(1, 1)),
        "wm1": np.ascontiguousarray(W_mlp1),
        "bm1": np.ascontiguousarray(b_mlp1.reshape(HID2)),
        "wm2": np.ascontiguousarray(W_mlp2),
        "bm2": np.ascontiguousarray(b_mlp2.reshape(1, NCLS)),
        "ident": ident,
    }
    in_maps = []
    for b in range(B):
        onehot = (label_ids[b][None, :] == lab_range[:, None]).astype(np.float32)
        m = dict(shared)
        m["feat"] = np.ascontiguousarray(feature[b].reshape(C, HW))
        m["onehot"] = np.ascontiguousarray(onehot)
        in_maps.append(m)

    trace = os.environ.get("BASS_KERNEL_TRACE", "") == "1"
    if trace:
        _maybe_install_trace_shim()
    res = run_bass_kernel_spmd(nc, in_maps, list(range(NCORES)), trace=trace,
                               tmpdir=os.environ.get("BASS_KERNEL_TMPDIR"))
    LAST_EXEC_NS = res.exec_time_ns
    if res.instructions_and_trace is not None:
        LAST_TRACE = res.instructions_and_trace[1]
    out = np.stack([res.results[i]["out"] for i in range(NCORES)], axis=0)
    return out.astype(np.float32)
